# revision 29
# baseline (speedup 1.0000x reference)
"""Trainium2 Bass kernel for nn_BlockDiagonalLayer.

Computes out[b, n*64+j] = sin(omega[n] * (sum_i x[b,n,i] * W[n,j,i] + bias[n,j]))
for B=2048, N=1024 networks, D_IN=D_OUT=64, sharded over 8 NeuronCores along N.

Device strategy (per core, 128 networks = 64 pairs):
  - weights pre-scaled by s1 = omega/(2pi) on host, split into bf16 hi+lo,
    packed into block-diagonal 128x128 lhsT tiles; x split into bf16 hi +
    fp8e4m3 lo (lo scaled by 16 to sit in fp8's normal range; the fp8 copy
    of Wh is pre-divided by 16 so the scales cancel exactly). Compensated
    product v = Wh@xh + (Wh/16)@(16*xl) + Wl@xh accumulates in PSUM; bf16/
    fp8 matmuls stream 1 col/cycle (4x faster than fp32). v = angle/(2pi)
    - s1*b.
  - sin range reduction in ONE fused custom-DVE op (4 ALU stages, magic-
    number round-to-int; ACT Sin table is valid only ~[-pi-0.16, pi+0.16]):
      q = (((v + s1*b) + MAGIC) - MAGIC) - v     = round(v + s1*b) - v
      y = Sin(-2pi*q + omega*b)                  (ScalarE)
    = sin(angle - 2pi*k).
  - output written fp16 (adds <=2.5e-4 abs error) stored transposed
    [pair, j2, b]; host transposes back and upcasts to f32.
Host does layout-only transforms + omega pre-scaling + hi/lo splits.
"""

import re

import numpy as np
import ml_dtypes

import concourse.bass as bass
import concourse.tile as tile
from concourse import bacc, mybir
from concourse import dve_ops as _dvo
from concourse.dve_ops import DveOp
from concourse.dve_spec import Spec, Src0, C0, C1
from concourse.bass_utils import run_bass_kernel_spmd

B, N, D = 2048, 1024, 64
NCORES = 8
NS = N // NCORES          # 128 nets per core
PAIRS = NS // 2           # 64
MMW = 512                 # matmul moving free dim (one PSUM bank)
EW = 2048                 # elementwise tile width (4 PSUM banks)

TWO_PI = float(2.0 * np.pi)
INV_2PI = float(1.0 / (2.0 * np.pi))
MAGIC = float(1.5 * 2 ** 23)

F32 = mybir.dt.float32
BF16 = mybir.dt.bfloat16
FP16 = mybir.dt.float16
FP8 = mybir.dt.float8e4
INT8 = mybir.dt.int8
XLS = 32.0                # power-of-2 scale applied to xl before fp8 cast

_REDSIN_NAME = "SIN_RANGE_REDUCE_K"


def _register_redsin() -> DveOp:
    """Register the fused range-reduction op q = round(v + s0) - v in the
    process-global custom-DVE table (idempotent)."""
    for op in _dvo.OPS:
        if op.name == _REDSIN_NAME:
            return op

    def _ref(in0, in1, s0, s1, imm2):
        f = np.float32
        t = (in0.astype(f) + f(s0)).astype(f)
        t = (t + f(s1)).astype(f)
        t = (t - f(s1)).astype(f)
        return (t - in0).astype(f)

    spec = Spec(body=(((Src0 + C0) + C1) - C1) - Src0, reference=_ref)
    op = DveOp(_REDSIN_NAME, spec, subdim=False, uops_sha={})
    _dvo.OPS.append(op)
    _dvo.CUSTOM_DVE_SPECS[op.name] = spec
    _dvo._SUB_OPCODE_FOR_NAME[op.name] = (
        _dvo._CUSTOM_DVE_ROW_BASE + len(_dvo.OPS) - 1)
    for ver in ("v3", "v4"):
        try:
            op.compile(ver)
        except ValueError as e:
            m = re.search(r'="([0-9a-f]+)"', str(e))
            op.uops_sha[ver] = m.group(1)
            op.compile(ver)
    return op


_REDSIN = _register_redsin()


def build_bass(repeat: int = 1):
    """Build the per-core Bass program (same NEFF on all 8 cores).

    repeat > 1 re-runs the whole main loop (idempotent writes) for timing.
    """
    nc = bacc.Bacc("TRN2", target_bir_lowering=False, debug=False,
                   num_devices=NCORES)
    xh_d = nc.dram_tensor("xh", [PAIRS, 128, B], BF16, kind="ExternalInput")
    xl_d = nc.dram_tensor("xl", [PAIRS, 128, B], BF16, kind="ExternalInput")
    wh_d = nc.dram_tensor("wh", [128, PAIRS * 128], BF16, kind="ExternalInput")
    wl_d = nc.dram_tensor("wl", [128, PAIRS * 128], BF16, kind="ExternalInput")
    sb_d = nc.dram_tensor("sb", [128, PAIRS], F32, kind="ExternalInput")
    ob_d = nc.dram_tensor("ob", [128, PAIRS], F32, kind="ExternalInput")
    yT_d = nc.dram_tensor("yT", [PAIRS, 128, B], INT8, kind="ExternalOutput")

    with tile.TileContext(nc) as tc:
        with (
            tc.tile_pool(name="aux", bufs=1) as aux_pool,
            tc.tile_pool(name="wconst", bufs=1) as wc_pool,
            tc.tile_pool(name="xin", bufs=4) as x_pool,
            tc.tile_pool(name="oout", bufs=3) as o_pool,
            tc.tile_pool(name="ew", bufs=3) as ew_pool,
            tc.tile_pool(name="ps", bufs=2, space="PSUM") as psum_pool,
        ):
            # --- constants (loaded once) ---
            wh_sb = wc_pool.tile([128, PAIRS * 128], BF16)
            wl_sb = wc_pool.tile([128, PAIRS * 128], BF16)
            for _c in range(8):
                _w = PAIRS * 128 // 8
                nc.scalar.dma_start(wh_sb[:, _c * _w:(_c + 1) * _w],
                                    wh_d[:, _c * _w:(_c + 1) * _w])
                nc.scalar.dma_start(wl_sb[:, _c * _w:(_c + 1) * _w],
                                    wl_d[:, _c * _w:(_c + 1) * _w])
            sb_sb = aux_pool.tile([128, PAIRS], F32)
            nc.gpsimd.dma_start(sb_sb[:], sb_d[:])
            ob_sb = aux_pool.tile([128, PAIRS], F32)
            nc.gpsimd.dma_start(ob_sb[:], ob_d[:])

            # --- main loop (optionally wrapped in a HW loop for timing) ---
            import contextlib
            rep_ctx = tc.For_i(0, repeat, 1) if repeat > 1 else contextlib.nullcontext()
            PB = 2  # pairs batched per DMA transfer
            with rep_ctx:
                for p0 in range(0, PAIRS, PB):
                    xht = x_pool.tile([128, PB * B], BF16, tag="xh")
                    nc.sync.dma_start(
                        xht[:].rearrange("p (a b) -> p a b", a=PB),
                        xh_d[p0:p0 + PB].rearrange("a p b -> p a b"))
                    xlt = x_pool.tile([128, PB * B], BF16, tag="xl")
                    nc.scalar.dma_start(
                        xlt[:].rearrange("p (a b) -> p a b", a=PB),
                        xl_d[p0:p0 + PB].rearrange("a p b -> p a b"))
                    outt = o_pool.tile([128, PB * B], INT8)
                    for a in range(PB):
                        p = p0 + a
                        wht = wh_sb[:, p * 128:(p + 1) * 128]
                        wlt = wl_sb[:, p * 128:(p + 1) * 128]
                        sbp = sb_sb[:, p:p + 1]
                        obp = ob_sb[:, p:p + 1]
                        for e in range(B // EW):
                            v = psum_pool.tile([128, EW], F32)
                            for h in range(EW // MMW):
                                lo = h * MMW
                                bcol = a * B + e * EW + lo
                                nc.tensor.matmul(
                                    v[:, lo:lo + MMW], wht,
                                    xht[:, bcol:bcol + MMW],
                                    start=True, stop=False)
                                nc.tensor.matmul(
                                    v[:, lo:lo + MMW], wht,
                                    xlt[:, bcol:bcol + MMW],
                                    start=False, stop=False)
                                nc.tensor.matmul(
                                    v[:, lo:lo + MMW], wlt,
                                    xht[:, bcol:bcol + MMW],
                                    start=False, stop=True)
                            q = ew_pool.tile([128, EW], F32, tag="ew")
                            nc.vector._custom_dve(
                                _REDSIN, out=q[:], in0=v[:],
                                s0=sbp, s1=MAGIC)
                            s16 = ew_pool.tile([128, EW], FP16, tag="s16")
                            nc.scalar.activation(
                                s16[:], q[:],
                                mybir.ActivationFunctionType.Sin,
                                bias=obp, scale=-TWO_PI)
                            nc.gpsimd.tensor_scalar_mul(
                                outt[:, a * B + e * EW:a * B + (e + 1) * EW],
                                s16[:], 127.0)
                    nc.scalar.dma_start(
                        yT_d[p0:p0 + PB].rearrange("a p b -> p a b"),
                        outt[:].rearrange("p (a b) -> p a b", a=PB))
    nc.compile()
    return nc


def prep_inputs(x, weights, bias, omega):
    """Host-side layout prep -> list of 8 per-core input dicts."""
    bf16 = ml_dtypes.bfloat16
    x3 = x.reshape(B, NCORES, NS, D)
    # xT_all[c, n, i, b] = x[b, c*128+n, i]; blocked for cache friendliness
    xT_all = np.empty((NCORES, NS, D, B), np.float32)
    BBLK = 128
    for b0 in range(0, B, BBLK):
        xT_all[:, :, :, b0:b0 + BBLK] = x3[b0:b0 + BBLK].transpose(1, 2, 3, 0)
    xT_all = xT_all.reshape(NCORES, PAIRS, 128, B)
    xh_all = xT_all.astype(bf16)
    xl_all = (xT_all - xh_all.astype(np.float32)).astype(bf16)

    s1_all = omega.astype(np.float64) * INV_2PI   # [N]

    in_maps = []
    for c in range(NCORES):
        sl = slice(c * NS, (c + 1) * NS)
        wc = weights[sl].astype(np.float64)    # [128, 64, 64] (j, i)
        s1c = s1_all[sl]                       # [128]
        wT = (wc * s1c[:, None, None]).transpose(0, 2, 1)  # [net, i, j] scaled
        # block-diagonal lhsT per pair: [i2, j2] with even net in the
        # upper-left 64x64 block and odd net in the lower-right
        bd = np.zeros((PAIRS, 128, 128), np.float32)
        bd[:, 0:D, 0:D] = wT[0::2]
        bd[:, D:, D:] = wT[1::2]
        bd_hi = bd.astype(bf16)
        bd_lo = (bd - bd_hi.astype(np.float32)).astype(bf16)
        wh_host = np.ascontiguousarray(
            bd_hi.transpose(1, 0, 2).reshape(128, PAIRS * 128))
        wl_host = np.ascontiguousarray(
            bd_lo.transpose(1, 0, 2).reshape(128, PAIRS * 128))

        bc = bias[sl].astype(np.float64)       # [128, 64]
        oc = omega[sl].astype(np.float64)      # [128]
        sb_c = (bc * s1c[:, None]).astype(np.float32)   # s1 * b
        ob_c = (bc * oc[:, None]).astype(np.float32)    # omega * b

        # pack [net, j] -> [j2 partition, pair]: partition row = parity*64+j
        def pack(m):
            r = np.empty((PAIRS, 128), np.float32)
            r[:, :D] = m[0::2]
            r[:, D:] = m[1::2]
            return np.ascontiguousarray(r.T)
        in_maps.append({
            "xh": np.ascontiguousarray(xh_all[c]),
            "xl": np.ascontiguousarray(xl_all[c]),
            "wh": wh_host, "wl": wl_host,
            "sb": pack(sb_c), "ob": pack(ob_c),
        })
    return in_maps


def assemble_output(results):
    """[8 cores] of yT int8 [PAIRS, 128, B] -> full f32 [B, N*D] (/127)."""
    out = np.empty((B, N * D), np.float32)
    for c in range(NCORES):
        yy = results[c]["yT"].reshape(NS * D, B)
        ov = out[:, c * NS * D:(c + 1) * NS * D]
        for b0 in range(0, B, 128):
            ov[b0:b0 + 128, :] = yy[:, b0:b0 + 128].T.astype(np.float32)
    out *= np.float32(1.0 / 127.0)
    return out


_NC_CACHE = {}


def kernel(x, weights, bias, omega):
    x = np.ascontiguousarray(x, np.float32)
    weights = np.ascontiguousarray(weights, np.float32)
    bias = np.ascontiguousarray(bias, np.float32)
    omega = np.ascontiguousarray(omega, np.float32)

    if "nc" not in _NC_CACHE:
        _NC_CACHE["nc"] = build_bass()
    nc = _NC_CACHE["nc"]
    in_maps = prep_inputs(x, weights, bias, omega)
    res = run_bass_kernel_spmd(nc, in_maps, core_ids=list(range(NCORES)))
    return assemble_output(res.results)


# revision 30
# speedup vs baseline: 7.0188x; 7.0188x over previous
"""Trainium2 Bass kernel for nn_BlockDiagonalLayer.

Computes out[b, n*64+j] = sin(omega[n] * (sum_i x[b,n,i] * W[n,j,i] + bias[n,j]))
for B=2048, N=1024 networks, D_IN=D_OUT=64, sharded over 8 NeuronCores along N.

Device strategy (per core, 128 networks = 64 pairs):
  - weights pre-scaled by s1 = omega/(2pi) on host, split into bf16 hi+lo,
    packed into block-diagonal 128x128 lhsT tiles; x split into bf16 hi +
    fp8e4m3 lo (lo scaled by 16 to sit in fp8's normal range; the fp8 copy
    of Wh is pre-divided by 16 so the scales cancel exactly). Compensated
    product v = Wh@xh + (Wh/16)@(16*xl) + Wl@xh accumulates in PSUM; bf16/
    fp8 matmuls stream 1 col/cycle (4x faster than fp32). v = angle/(2pi)
    - s1*b.
  - sin range reduction in ONE fused custom-DVE op (4 ALU stages, magic-
    number round-to-int; ACT Sin table is valid only ~[-pi-0.16, pi+0.16]):
      q = (((v + s1*b) + MAGIC) - MAGIC) - v     = round(v + s1*b) - v
      y = Sin(-2pi*q + omega*b)                  (ScalarE)
    = sin(angle - 2pi*k).
  - output written fp16 (adds <=2.5e-4 abs error) stored transposed
    [pair, j2, b]; host transposes back and upcasts to f32.
Host does layout-only transforms + omega pre-scaling + hi/lo splits.
"""

import re

import numpy as np
import ml_dtypes

import concourse.bass as bass
import concourse.tile as tile
from concourse import bacc, mybir
from concourse import dve_ops as _dvo
from concourse.dve_ops import DveOp
from concourse.dve_spec import Spec, Src0, C0, C1
from concourse.bass_utils import run_bass_kernel_spmd

B, N, D = 2048, 1024, 64
NCORES = 8
NS = N // NCORES          # 128 nets per core
PAIRS = NS // 2           # 64
MMW = 512                 # matmul moving free dim (one PSUM bank)
EW = 2048                 # elementwise tile width (4 PSUM banks)

TWO_PI = float(2.0 * np.pi)
INV_2PI = float(1.0 / (2.0 * np.pi))
MAGIC = float(1.5 * 2 ** 23)

F32 = mybir.dt.float32
BF16 = mybir.dt.bfloat16
FP16 = mybir.dt.float16
FP8 = mybir.dt.float8e4
INT8 = mybir.dt.int8
XLS = 16.0                # power-of-2 scale applied to xl before fp8 cast

_REDSIN_NAME = "SIN_RANGE_REDUCE_K"


def _register_redsin() -> DveOp:
    """Register the fused range-reduction op q = round(v + s0) - v in the
    process-global custom-DVE table (idempotent)."""
    for op in _dvo.OPS:
        if op.name == _REDSIN_NAME:
            return op

    def _ref(in0, in1, s0, s1, imm2):
        f = np.float32
        t = (in0.astype(f) + f(s0)).astype(f)
        t = (t + f(s1)).astype(f)
        t = (t - f(s1)).astype(f)
        return (t - in0).astype(f)

    spec = Spec(body=(((Src0 + C0) + C1) - C1) - Src0, reference=_ref)
    op = DveOp(_REDSIN_NAME, spec, subdim=False, uops_sha={})
    _dvo.OPS.append(op)
    _dvo.CUSTOM_DVE_SPECS[op.name] = spec
    _dvo._SUB_OPCODE_FOR_NAME[op.name] = (
        _dvo._CUSTOM_DVE_ROW_BASE + len(_dvo.OPS) - 1)
    for ver in ("v3", "v4"):
        try:
            op.compile(ver)
        except ValueError as e:
            m = re.search(r'="([0-9a-f]+)"', str(e))
            op.uops_sha[ver] = m.group(1)
            op.compile(ver)
    return op


_REDSIN = _register_redsin()


def build_bass(repeat: int = 1):
    """Build the per-core Bass program (same NEFF on all 8 cores).

    repeat > 1 re-runs the whole main loop (idempotent writes) for timing.
    """
    nc = bacc.Bacc("TRN2", target_bir_lowering=False, debug=False,
                   num_devices=NCORES)
    xh_d = nc.dram_tensor("xh", [PAIRS, 128, B], BF16, kind="ExternalInput")
    xl_d = nc.dram_tensor("xl", [PAIRS, 128, B], FP8, kind="ExternalInput")
    wh_d = nc.dram_tensor("wh", [128, PAIRS * 128], BF16, kind="ExternalInput")
    wl_d = nc.dram_tensor("wl", [128, PAIRS * 128], BF16, kind="ExternalInput")
    w8_d = nc.dram_tensor("w8", [128, PAIRS * 128], FP8, kind="ExternalInput")
    sb_d = nc.dram_tensor("sb", [128, PAIRS], F32, kind="ExternalInput")
    ob_d = nc.dram_tensor("ob", [128, PAIRS], F32, kind="ExternalInput")
    yT_d = nc.dram_tensor("yT", [PAIRS, 128, B], FP16, kind="ExternalOutput")

    with tile.TileContext(nc) as tc:
        with (
            tc.tile_pool(name="aux", bufs=1) as aux_pool,
            tc.tile_pool(name="wconst", bufs=1) as wc_pool,
            tc.tile_pool(name="xin", bufs=4) as x_pool,
            tc.tile_pool(name="oout", bufs=3) as o_pool,
            tc.tile_pool(name="ew", bufs=3) as ew_pool,
            tc.tile_pool(name="ps", bufs=2, space="PSUM") as psum_pool,
        ):
            # --- constants (loaded once) ---
            wh_sb = wc_pool.tile([128, PAIRS * 128], BF16)
            wl_sb = wc_pool.tile([128, PAIRS * 128], BF16)
            for _c in range(8):
                _w = PAIRS * 128 // 8
                nc.scalar.dma_start(wh_sb[:, _c * _w:(_c + 1) * _w],
                                    wh_d[:, _c * _w:(_c + 1) * _w])
                nc.scalar.dma_start(wl_sb[:, _c * _w:(_c + 1) * _w],
                                    wl_d[:, _c * _w:(_c + 1) * _w])
            w8_sb = wc_pool.tile([128, PAIRS * 128], FP8)
            nc.gpsimd.dma_start(w8_sb[:], w8_d[:])
            sb_sb = aux_pool.tile([128, PAIRS], F32)
            nc.gpsimd.dma_start(sb_sb[:], sb_d[:])
            ob_sb = aux_pool.tile([128, PAIRS], F32)
            nc.gpsimd.dma_start(ob_sb[:], ob_d[:])

            # --- main loop (optionally wrapped in a HW loop for timing) ---
            import contextlib
            rep_ctx = tc.For_i(0, repeat, 1) if repeat > 1 else contextlib.nullcontext()
            PB = 2  # pairs batched per DMA transfer
            with rep_ctx:
                for p0 in range(0, PAIRS, PB):
                    xht = x_pool.tile([128, PB * B], BF16, tag="xh")
                    nc.sync.dma_start(
                        xht[:].rearrange("p (a b) -> p a b", a=PB),
                        xh_d[p0:p0 + PB].rearrange("a p b -> p a b"))
                    xlt = x_pool.tile([128, PB * B], FP8, tag="xl")
                    nc.scalar.dma_start(
                        xlt[:].rearrange("p (a b) -> p a b", a=PB),
                        xl_d[p0:p0 + PB].rearrange("a p b -> p a b"))
                    outt = o_pool.tile([128, PB * B], FP16)
                    for a in range(PB):
                        p = p0 + a
                        wht = wh_sb[:, p * 128:(p + 1) * 128]
                        wlt = wl_sb[:, p * 128:(p + 1) * 128]
                        w8t = w8_sb[:, p * 128:(p + 1) * 128]
                        sbp = sb_sb[:, p:p + 1]
                        obp = ob_sb[:, p:p + 1]
                        for e in range(B // EW):
                            v = psum_pool.tile([128, EW], F32)
                            for h in range(EW // MMW):
                                lo = h * MMW
                                bcol = a * B + e * EW + lo
                                nc.tensor.matmul(
                                    v[:, lo:lo + MMW], wht,
                                    xht[:, bcol:bcol + MMW],
                                    start=True, stop=False)
                                nc.tensor.matmul(
                                    v[:, lo:lo + MMW], w8t,
                                    xlt[:, bcol:bcol + MMW],
                                    start=False, stop=False)
                                nc.tensor.matmul(
                                    v[:, lo:lo + MMW], wlt,
                                    xht[:, bcol:bcol + MMW],
                                    start=False, stop=True)
                            q = ew_pool.tile([128, EW], F32, tag="ew")
                            nc.vector._custom_dve(
                                _REDSIN, out=q[:], in0=v[:],
                                s0=sbp, s1=MAGIC)
                            nc.scalar.activation(
                                outt[:, a * B + e * EW:a * B + (e + 1) * EW], q[:],
                                mybir.ActivationFunctionType.Sin,
                                bias=obp, scale=-TWO_PI)
                    nc.gpsimd.dma_start(
                        yT_d[p0:p0 + PB].rearrange("a p b -> p a b"),
                        outt[:].rearrange("p (a b) -> p a b", a=PB))
    nc.compile()
    return nc


def prep_inputs(x, weights, bias, omega):
    """Host-side layout prep -> list of 8 per-core input dicts."""
    bf16 = ml_dtypes.bfloat16
    x3 = x.reshape(B, NCORES, NS, D)
    # xT_all[c, n, i, b] = x[b, c*128+n, i]; blocked for cache friendliness
    xT_all = np.empty((NCORES, NS, D, B), np.float32)
    BBLK = 128
    for b0 in range(0, B, BBLK):
        xT_all[:, :, :, b0:b0 + BBLK] = x3[b0:b0 + BBLK].transpose(1, 2, 3, 0)
    xT_all = xT_all.reshape(NCORES, PAIRS, 128, B)
    xh_all = xT_all.astype(bf16)
    # residual scaled by XLS so it lands in fp8e4m3's normal range
    xl_all = ((xT_all - xh_all.astype(np.float32)) * XLS).astype(
        ml_dtypes.float8_e4m3)

    s1_all = omega.astype(np.float64) * INV_2PI   # [N]

    in_maps = []
    for c in range(NCORES):
        sl = slice(c * NS, (c + 1) * NS)
        wc = weights[sl].astype(np.float64)    # [128, 64, 64] (j, i)
        s1c = s1_all[sl]                       # [128]
        wT = (wc * s1c[:, None, None]).transpose(0, 2, 1)  # [net, i, j] scaled
        # block-diagonal lhsT per pair: [i2, j2] with even net in the
        # upper-left 64x64 block and odd net in the lower-right
        bd = np.zeros((PAIRS, 128, 128), np.float32)
        bd[:, 0:D, 0:D] = wT[0::2]
        bd[:, D:, D:] = wT[1::2]
        bd_hi = bd.astype(bf16)
        bd_lo = (bd - bd_hi.astype(np.float32)).astype(bf16)
        wh_host = np.ascontiguousarray(
            bd_hi.transpose(1, 0, 2).reshape(128, PAIRS * 128))
        wl_host = np.ascontiguousarray(
            bd_lo.transpose(1, 0, 2).reshape(128, PAIRS * 128))

        # fp8 copy of Wh descaled by XLS for the (XLS*xl) fp8 matmul
        w8_host = np.ascontiguousarray(
            (bd_hi.astype(np.float32) / XLS).astype(ml_dtypes.float8_e4m3)
            .transpose(1, 0, 2).reshape(128, PAIRS * 128))

        bc = bias[sl].astype(np.float64)       # [128, 64]
        oc = omega[sl].astype(np.float64)      # [128]
        sb_c = (bc * s1c[:, None]).astype(np.float32)   # s1 * b
        ob_c = (bc * oc[:, None]).astype(np.float32)    # omega * b

        # pack [net, j] -> [j2 partition, pair]: partition row = parity*64+j
        def pack(m):
            r = np.empty((PAIRS, 128), np.float32)
            r[:, :D] = m[0::2]
            r[:, D:] = m[1::2]
            return np.ascontiguousarray(r.T)
        in_maps.append({
            "xh": np.ascontiguousarray(xh_all[c]),
            "xl": np.ascontiguousarray(xl_all[c]),
            "wh": wh_host, "wl": wl_host, "w8": w8_host,
            "sb": pack(sb_c), "ob": pack(ob_c),
        })
    return in_maps


def assemble_output(results):
    """[8 cores] of yT fp16 [PAIRS, 128, B] -> full f32 [B, N*D]."""
    out = np.empty((B, N * D), np.float32)
    for c in range(NCORES):
        yy = results[c]["yT"].reshape(NS * D, B)
        ov = out[:, c * NS * D:(c + 1) * NS * D]
        for b0 in range(0, B, 128):
            ov[b0:b0 + 128, :] = yy[:, b0:b0 + 128].T.astype(np.float32)
    return out


_NC_CACHE = {}


def kernel(x, weights, bias, omega):
    x = np.ascontiguousarray(x, np.float32)
    weights = np.ascontiguousarray(weights, np.float32)
    bias = np.ascontiguousarray(bias, np.float32)
    omega = np.ascontiguousarray(omega, np.float32)

    if "nc" not in _NC_CACHE:
        _NC_CACHE["nc"] = build_bass()
    nc = _NC_CACHE["nc"]
    in_maps = prep_inputs(x, weights, bias, omega)
    res = run_bass_kernel_spmd(nc, in_maps, core_ids=list(range(NCORES)))
    return assemble_output(res.results)
